# revision 8
# baseline (speedup 1.0000x reference)
"""Two-layer GCN (nn_Method_GCN_11098195493080) as a Bass/Tile kernel on 8
Trainium2 NeuronCores.

Strategy (follows the 1D graph-partition sharding hint):
  - Nodes sharded 8 ways; edges owned by the destination core.
  - Layer 1: y = dinv * (x_shard @ W1) on PE (bf16), AllGather -> full
    message table in every core's HBM (the halo exchange).
  - Aggregation (both layers): batched dma_gather fetches 256B bf16 rows
    edge-major; a one-hot ST[128 edges, 128 dst] built on the vector engine
    turns the segment sum into PE matmuls accumulated in PSUM. Self loops
    are added densely from the local shard. The 4 gather streams (one per
    int16-reachable source-table chunk) run on 4 SWDGE queues, i.e. on all
    8 GPSIMD cores in parallel.
  - Layer 2 aggregates h *before* applying W2 (propagation commutes with
    the weight multiply); the @W2 + bias + log_softmax runs per-tile.
  - Each core permutes its nodes into 100 tiles of 128 slots balancing
    per-(tile, chunk) edge counts, so nearly every group packs into 4
    blocks of 128 gathered slots (minimal padding, uniform SPMD streams).
  - Host-side work is integer graph partitioning (shard, bucket, balance,
    bincount for degrees); all float math runs on the NeuronCores.
"""

import heapq
import numpy as np
import ml_dtypes

import concourse.bass as bass
import concourse.bacc as bacc
import concourse.mybir as mybir
import concourse.tile as tile
from concourse import bass_utils
from concourse.masks import make_identity

F32 = mybir.dt.float32
BF16 = mybir.dt.bfloat16
I16 = mybir.dt.int16
AF = mybir.ActivationFunctionType
OP = mybir.AluOpType
NPBF16 = ml_dtypes.bfloat16

N_CORES = 8
N_CHUNKS = 4          # source-table chunks (int16 gather index reach)
P = 128               # partitions / dst-tile size
PAD_DST = 254.0       # dst_local value whose one-hot row is all-zero
SLOT_CAP = 6144       # max gathered edge slots per supertile buffer
TROW = 128            # table row width (bf16): hid data + zero pad = 256B


def _balance_core(d4, nt, cap_nodes=P):
    """Assign nodes (rows of d4 = per-chunk in-degree) to nt tiles,
    balancing per-(tile, chunk) edge loads. Returns slot array [nt*P]
    holding node ids (-1 = empty)."""
    n = d4.shape[0]
    order = np.argsort(-d4.sum(axis=1), kind="stable")
    loads = np.zeros((nt, N_CHUNKS), dtype=np.int64)
    counts = np.zeros(nt, dtype=np.int64)
    slot_of = np.empty(n, dtype=np.int64)
    full_penalty = np.zeros(nt, dtype=np.int64)
    for node in order:
        d = d4[node]
        score = (loads + d).max(axis=1) + full_penalty
        t = int(np.argmin(score))
        loads[t] += d
        slot_of[node] = t * cap_nodes + counts[t]
        counts[t] += 1
        if counts[t] >= cap_nodes:
            full_penalty[t] = 1 << 40
    slots = np.full(nt * cap_nodes, -1, dtype=np.int64)
    slots[slot_of] = np.arange(n)
    return slot_of, slots


class Plan:
    """Static, core-uniform schedule derived from the (integer) graph."""

    def __init__(self, n_nodes, fin, hid, fout, edge_index, n_cores=N_CORES,
                 nt=None):
        assert n_nodes % n_cores == 0
        self.n_nodes = n_nodes
        self.fin, self.hid, self.fout = fin, hid, fout
        self.n_cores = n_cores
        self.base = n_nodes // n_cores
        min_nt = (self.base + P - 1) // P
        self.nt = nt if nt is not None else min_nt + 2 + (min_nt + 63) // 64
        assert self.nt * P >= self.base
        self.nsh = self.nt * P
        self.ntab = self.nsh * n_cores
        assert self.ntab % N_CHUNKS == 0
        self.ch = self.ntab // N_CHUNKS
        assert self.ch <= 32767, "chunk must fit int16 gather index"
        assert fin % P == 0
        self.kch = fin // P

        src = np.asarray(edge_index[0], dtype=np.int64)
        dst = np.asarray(edge_index[1], dtype=np.int64)
        s_owner = src // self.base
        d_owner = dst // self.base
        chunk = s_owner // (n_cores // N_CHUNKS)

        # per-core node balance on per-chunk in-degree
        self.slot_of = np.empty((n_cores, self.base), dtype=np.int64)
        for c in range(n_cores):
            sel = d_owner == c
            dl = dst[sel] - c * self.base
            ck = chunk[sel]
            d4 = np.zeros((self.base, N_CHUNKS), dtype=np.int64)
            np.add.at(d4, (dl, ck), 1)
            slot_of, _ = _balance_core(d4, self.nt)
            self.slot_of[c] = slot_of

        # slot-space edge endpoints
        d_slot = self.slot_of[d_owner, dst - d_owner * self.base]
        gid_src = s_owner * self.nsh + self.slot_of[
            s_owner, src - s_owner * self.base
        ]
        idx_local = gid_src - chunk * self.ch
        tile_id = d_slot // P
        dloc = (d_slot % P).astype(np.float32)

        key = (d_owner * self.nt + tile_id) * N_CHUNKS + chunk
        order = np.argsort(key, kind="stable")
        self._sorted_idx = idx_local[order]
        self._sorted_dloc = dloc[order]
        ngroups = n_cores * self.nt * N_CHUNKS
        sizes = np.bincount(key, minlength=ngroups).reshape(
            n_cores, self.nt, N_CHUNKS
        )
        self._gstart = np.zeros(ngroups + 1, dtype=np.int64)
        np.cumsum(sizes.reshape(-1), out=self._gstart[1:])
        self.sizes = sizes

        nb = (sizes.max(axis=0) + P - 1) // P             # [nt, N_CHUNKS]
        nb[:, 0] = np.maximum(nb[:, 0], 1)
        self.nb = nb
        self.bp = np.zeros((N_CHUNKS, self.nt + 1), dtype=np.int64)
        np.cumsum(nb.T, axis=1, out=self.bp[:, 1:])
        self.blocks_c = self.bp[:, -1].copy()
        self.slots_c = self.blocks_c * P
        self.total_slots = int(self.slots_c.sum())

        self.supertiles = []
        t0 = 0
        while t0 < self.nt:
            t1 = t0 + 1
            while t1 < self.nt:
                tot = int(nb[t0:t1 + 1].sum()) * P
                if tot > SLOT_CAP:
                    break
                t1 += 1
            self.supertiles.append((t0, t1))
            t0 = t1
        self.max_sg_blocks = max(
            int(self.nb[a:b].sum()) for a, b in self.supertiles
        )

        # degrees (with self loop), per core wrapped [128, nt], slot order
        deg = np.bincount(dst, minlength=n_nodes).astype(np.float32) + 1.0
        self.degw = np.ones((n_cores, P, self.nt), dtype=np.float32)
        for c in range(n_cores):
            d = np.ones(self.nsh, dtype=np.float32)
            d[self.slot_of[c]] = deg[c * self.base:(c + 1) * self.base]
            self.degw[c] = d.reshape(self.nt, P).T

        # per-core gather index / dst_local arrays in slot order
        self.idx16 = []
        self.dstl = []
        for c in range(n_cores):
            idx_c, dstl_c = [], []
            for k in range(N_CHUNKS):
                s = int(self.slots_c[k])
                ia = np.zeros(s, dtype=np.int16)
                da = np.full(s, PAD_DST, dtype=np.float32)
                for t in range(self.nt):
                    g = (c * self.nt + t) * N_CHUNKS + k
                    a, b = self._gstart[g], self._gstart[g + 1]
                    o = int(self.bp[k][t]) * P
                    n = int(b - a)
                    ia[o:o + n] = self._sorted_idx[a:b].astype(np.int16)
                    da[o:o + n] = self._sorted_dloc[a:b]
                idx_c.append(np.ascontiguousarray(
                    np.tile(ia.reshape(-1, 16).T, (P // 16, 1))
                ).astype(np.int16))
                da16 = da.astype(np.int16)
                dstl_c.append(np.ascontiguousarray(
                    np.tile(da16.reshape(-1, 16).T, (P // 16, 1))
                ).astype(np.int16))
            self.idx16.append(idx_c)
            self.dstl.append(dstl_c)


def build_nc(plan: Plan):
    nc = bacc.Bacc(
        "TRN2",
        target_bir_lowering=False,
        debug=False,
        enable_asserts=False,
        num_devices=plan.n_cores,
        num_swdge_queues=N_CHUNKS,
    )
    fin, hid, fout = plan.fin, plan.hid, plan.fout
    nt, nsh, kch = plan.nt, plan.nsh, plan.kch

    xT = nc.dram_tensor("xT", [fin, nsh], BF16, kind="ExternalInput")
    degw = nc.dram_tensor("degw", [P, nt], F32, kind="ExternalInput")
    w1 = nc.dram_tensor("w1", [fin, hid], BF16, kind="ExternalInput")
    b1 = nc.dram_tensor("b1", [1, hid], F32, kind="ExternalInput")
    w2 = nc.dram_tensor("w2", [hid, fout], F32, kind="ExternalInput")
    b2 = nc.dram_tensor("b2", [1, fout], F32, kind="ExternalInput")
    idx_d = [
        nc.dram_tensor(f"idx{c}", [P, int(plan.slots_c[c]) // 16], I16,
                       kind="ExternalInput")
        for c in range(N_CHUNKS)
    ]
    dstl_d = [
        nc.dram_tensor(f"dstl{c}", [P, int(plan.slots_c[c]) // 16], I16,
                       kind="ExternalInput")
        for c in range(N_CHUNKS)
    ]
    onehot = nc.dram_tensor("onehot", [256, TROW], BF16,
                            kind="ExternalInput")
    out = nc.dram_tensor("out", [nsh, fout], F32, kind="ExternalOutput")

    rg = [list(range(plan.n_cores))]

    with tile.TileContext(nc) as tc:
        with (
            tc.tile_pool(name="const", bufs=1) as cp,
            tc.tile_pool(name="dram", bufs=1, space="DRAM") as dp,
        ):
            # ---- constants -------------------------------------------------
            ident = cp.tile([P, P], F32)
            make_identity(nc, ident[:])

            w1sb = cp.tile([P, kch, hid], BF16)
            nc.sync.dma_start(
                w1sb[:], w1.ap().rearrange("(a p) f -> p a f", p=P)
            )
            w2sb = cp.tile([hid, fout], F32)
            nc.sync.dma_start(w2sb[:], w2.ap())
            b1row = cp.tile([P, hid], F32)
            nc.sync.dma_start(b1row[:], b1.ap().to_broadcast([P, hid]))
            b2row = cp.tile([P, fout], F32)
            nc.sync.dma_start(b2row[:], b2.ap().to_broadcast([P, fout]))

            degt = cp.tile([P, nt], F32)
            nc.sync.dma_start(degt[:], degw.ap())
            rec = cp.tile([P, nt], F32)
            nc.vector.reciprocal(rec[:], degt[:])
            dinv = cp.tile([P, nt], F32)
            nc.scalar.activation(dinv[:], rec[:], AF.Sqrt)

            idxsb = []
            dstlsb = []
            for c in range(N_CHUNKS):
                it = cp.tile([P, int(plan.slots_c[c]) // 16], I16,
                             tag=f"idx{c}")
                nc.sync.dma_start(it[:], idx_d[c].ap())
                idxsb.append(it)
                dt_ = cp.tile([P, int(plan.slots_c[c]) // 16], I16,
                              tag=f"dstl{c}")
                nc.sync.dma_start(dt_[:], dstl_d[c].ap())
                dstlsb.append(dt_)

            # local table shards (row = TROW bf16: hid data + zero pad)
            w_loc = cp.tile([P, nt, hid], F32)     # dinv*y + b1
            m_all = cp.tile([P, nt], F32)
            ssum_all = cp.tile([P, nt], F32)
            y_loc = cp.tile([P, nt, TROW], BF16)
            z_loc = cp.tile([P, nt, TROW], BF16)
            nc.vector.memset(y_loc[:], 0.0)
            nc.vector.memset(z_loc[:], 0.0)
            out_loc = cp.tile([P, nt, fout], F32)

            y_bounce = dp.tile([nsh, TROW], BF16)
            h_bounce = dp.tile([nsh, TROW], BF16)
            table1 = nc.dram_tensor("table1", [plan.ntab, TROW], BF16,
                                    kind="Internal", addr_space="Shared")
            table2 = nc.dram_tensor("table2", [plan.ntab, TROW], BF16,
                                    kind="Internal", addr_space="Shared")

            # ---- phase 1: y = dinv * (x @ W1), AllGather -> table1 ---------
            WB = 8
            with (
                tc.tile_pool(name="xload", bufs=2) as xp,
                tc.tile_pool(name="ps1", bufs=4, space="PSUM") as pp1,
            ):
                xTap = xT.ap().rearrange("(a p) n -> p a n", p=P)
                for wb in range(0, nt, WB):
                    nwin = min(WB, nt - wb)
                    xt = xp.tile([P, kch, P * WB], BF16, tag="xt")
                    nc.sync.dma_start(
                        xt[:, :, : P * nwin],
                        xTap[:, :, wb * P:(wb + nwin) * P],
                    )
                    for w in range(nwin):
                        t = wb + w
                        ps = pp1.tile([P, hid], F32, tag="ps1")
                        for a in range(kch):
                            nc.tensor.matmul(
                                ps[:],
                                lhsT=xt[:, a, w * P:(w + 1) * P],
                                rhs=w1sb[:, a, :],
                                start=(a == 0),
                                stop=(a == kch - 1),
                            )
                        nc.vector.tensor_scalar(
                            out=y_loc[:, t, :hid], in0=ps[:],
                            scalar1=dinv[:, t:t + 1], scalar2=None,
                            op0=OP.mult,
                        )
                        nc.vector.scalar_tensor_tensor(
                            out=w_loc[:, t, :], in0=y_loc[:, t, :hid],
                            scalar=dinv[:, t:t + 1], in1=b1row[:],
                            op0=OP.mult, op1=OP.add,
                        )
            nc.sync.dma_start(
                y_bounce[:].rearrange("(t p) f -> p t f", p=P), y_loc[:]
            )
            nc.gpsimd.collective_compute(
                "AllGather", OP.bypass, replica_groups=rg,
                ins=[y_bounce.opt()], outs=[table1.ap()],
            )

            # ---- aggregation pass (both layers) ----------------------------
            def aggregate(table, epilogue):
                with (
                    tc.tile_pool(name="gath", bufs=2) as gp,
                    tc.tile_pool(name="stp", bufs=4) as stp,
                    tc.tile_pool(name="ps2", bufs=3, space="PSUM") as pp2,
                    tc.tile_pool(name="eps", bufs=3) as ep,
                    tc.tile_pool(name="psT", bufs=2, space="PSUM") as ppT,
                    tc.tile_pool(name="pso", bufs=2, space="PSUM") as ppo,
                ):
                    for (t0, t1) in plan.supertiles:
                        off = {}
                        yb = gp.tile([P, plan.max_sg_blocks, TROW], BF16,
                                     tag="yb")
                        sb = gp.tile([P, plan.max_sg_blocks, TROW], BF16,
                                     tag="sb")
                        o = 0
                        for c in range(N_CHUNKS):
                            blk0 = int(plan.bp[c][t0])
                            blk1 = int(plan.bp[c][t1])
                            nbg = blk1 - blk0
                            off[c] = (o, blk0)
                            if nbg == 0:
                                continue
                            nc.gpsimd.dma_gather(
                                yb[:, o:o + nbg, :],
                                table.ap()[c * plan.ch:(c + 1) * plan.ch, :],
                                idxsb[c][:, blk0 * 8:blk1 * 8],
                                nbg * P,
                                nbg * P,
                                TROW,
                                single_packet=False,
                                queue_num=c,
                            )
                            nc.gpsimd.dma_gather(
                                sb[:, o:o + nbg, :],
                                onehot.ap(),
                                dstlsb[c][:, blk0 * 8:blk1 * 8],
                                nbg * P,
                                nbg * P,
                                TROW,
                                single_packet=False,
                                queue_num=c,
                            )
                            o += nbg
                        for t in range(t0, t1):
                            ps = pp2.tile([P, hid], F32, tag="ps2")
                            total = int(plan.nb[t].sum())
                            done = 0
                            for c in range(N_CHUNKS):
                                o, blk0 = off[c]
                                for b in range(int(plan.bp[c][t]),
                                               int(plan.bp[c][t + 1])):
                                    j = o + (b - blk0)
                                    nc.tensor.matmul(
                                        ps[:], lhsT=sb[:, j, :],
                                        rhs=yb[:, j, :hid],
                                        start=(done == 0),
                                        stop=(done == total - 1),
                                    )
                                    done += 1
                            epilogue(t, ps, ep, ppT, ppo)

            # ---- layer-1 epilogue: z = dinv*relu(dinv*(s+y) + b1) ----------
            def epi1(t, ps, ep, ppT, ppo):
                a2 = ep.tile([P, hid], F32, tag="a2")
                nc.vector.scalar_tensor_tensor(
                    out=a2[:], in0=ps[:], scalar=dinv[:, t:t + 1],
                    in1=w_loc[:, t, :], op0=OP.mult, op1=OP.add,
                )
                nc.vector.tensor_scalar(
                    out=z_loc[:, t, :hid], in0=a2[:],
                    scalar1=dinv[:, t:t + 1], scalar2=0.0,
                    op0=OP.mult, op1=OP.max,
                )

            aggregate(table1, epi1)
            nc.sync.dma_start(
                h_bounce[:].rearrange("(t p) f -> p t f", p=P), z_loc[:]
            )
            nc.gpsimd.collective_compute(
                "AllGather", OP.bypass, replica_groups=rg,
                ins=[h_bounce.opt()], outs=[table2.ap()],
            )

            # ---- layer-2 epilogue: log_softmax(dinv*(s+z) @ W2 + b2) -------
            def epi2(t, ps, ep, ppT, ppo):
                u = ep.tile([P, hid], F32, tag="u")
                nc.vector.tensor_tensor(
                    out=u[:], in0=ps[:], in1=z_loc[:, t, :hid], op=OP.add
                )
                opre = ep.tile([P, hid], F32, tag="a1")
                nc.vector.tensor_scalar(
                    out=opre[:], in0=u[:], scalar1=dinv[:, t:t + 1],
                    scalar2=None, op0=OP.mult,
                )
                pT = ppT.tile([hid, P], F32, tag="pT")
                nc.tensor.transpose(out=pT[:], in_=opre[:],
                                    identity=ident[:])
                opT = ep.tile([hid, P], F32, tag="opT")
                nc.scalar.copy(opT[:], pT[:])
                po = ppo.tile([P, fout], F32, tag="po")
                nc.tensor.matmul(po[:], lhsT=opT[:], rhs=w2sb[:],
                                 start=True, stop=True)
                nc.vector.tensor_tensor(
                    out=out_loc[:, t, :], in0=po[:], in1=b2row[:], op=OP.add
                )
                nc.vector.reduce_max(m_all[:, t:t + 1], out_loc[:, t, :],
                                     axis=mybir.AxisListType.X, negate=True)
                e = ep.tile([P, fout], F32, tag="e")
                nc.scalar.activation(e[:], out_loc[:, t, :], AF.Exp,
                                     bias=m_all[:, t:t + 1],
                                     accum_out=ssum_all[:, t:t + 1])

            aggregate(table2, epi2)
            # deferred log-sum-exp: out -= log(ssum) - m_all (m_all = -max)
            lse_all = cp.tile([P, nt], F32)
            nc.scalar.activation(lse_all[:], ssum_all[:], AF.Ln)
            c_all = cp.tile([P, nt], F32)
            nc.vector.tensor_tensor(
                out=c_all[:], in0=lse_all[:], in1=m_all[:], op=OP.subtract
            )
            with tc.tile_pool(name="fin", bufs=4) as fp:
                for t in range(nt):
                    ot = fp.tile([P, fout], F32, tag="ot")
                    nc.vector.tensor_scalar(
                        out=ot[:], in0=out_loc[:, t, :],
                        scalar1=c_all[:, t:t + 1], scalar2=None,
                        op0=OP.subtract,
                    )
                    nc.vector.tensor_copy(out_loc[:, t, :], ot[:])
            nc.sync.dma_start(
                out.ap().rearrange("(t p) f -> p t f", p=P), out_loc[:]
            )

    nc.compile()
    return nc


def make_in_maps(plan: Plan, x, W1, b1, W2, b2):
    x = np.asarray(x, dtype=np.float32)
    w1b = np.ascontiguousarray(W1, dtype=np.float32).astype(NPBF16)
    in_maps = []
    for c in range(plan.n_cores):
        xT = np.zeros((plan.fin, plan.nsh), dtype=NPBF16)
        xs = x[c * plan.base:(c + 1) * plan.base, :].astype(NPBF16)
        xT[:, plan.slot_of[c]] = xs.T
        m = {
            "xT": xT,
            "degw": plan.degw[c],
            "w1": w1b,
            "b1": np.asarray(b1, dtype=np.float32).reshape(1, -1),
            "w2": np.ascontiguousarray(W2, dtype=np.float32),
            "b2": np.asarray(b2, dtype=np.float32).reshape(1, -1),
        }
        onehot = np.zeros((256, TROW), dtype=NPBF16)
        onehot[:P, :P] = np.eye(P, dtype=np.float32).astype(NPBF16)
        m["onehot"] = onehot
        for k in range(N_CHUNKS):
            m[f"idx{k}"] = plan.idx16[c][k]
            m[f"dstl{k}"] = plan.dstl[c][k]
        in_maps.append(m)
    return in_maps


_CACHE = {}


def _get_compiled(n_nodes, fin, hid, fout, edge_key, edge_index):
    key = (n_nodes, fin, hid, fout, edge_key)
    if key not in _CACHE:
        plan = Plan(n_nodes, fin, hid, fout, edge_index)
        nc = build_nc(plan)
        _CACHE[key] = (plan, nc)
    return _CACHE[key]


def kernel(x, edge_index, W1, b1, W2, b2, _trace=False):
    x = np.asarray(x)
    edge_index = np.asarray(edge_index)
    n_nodes, fin = x.shape
    hid = np.asarray(W1).shape[1]
    fout = np.asarray(W2).shape[1]
    edge_key = hash(edge_index.tobytes())
    plan, nc = _get_compiled(n_nodes, fin, hid, fout, edge_key, edge_index)
    in_maps = make_in_maps(plan, x, W1, b1, W2, b2)
    res = bass_utils.run_bass_kernel_spmd(
        nc, in_maps, core_ids=list(range(plan.n_cores)), trace=_trace
    )
    parts = [
        res.results[c]["out"][plan.slot_of[c], :]
        for c in range(plan.n_cores)
    ]
    out = np.concatenate(parts, axis=0).astype(np.float32)
    kernel.last_results = res
    return out


# revision 9
# speedup vs baseline: 1.5641x; 1.5641x over previous
"""Two-layer GCN (nn_Method_GCN_11098195493080) as a Bass/Tile kernel on 8
Trainium2 NeuronCores.

Strategy (follows the 1D graph-partition sharding hint):
  - Nodes sharded 8 ways; edges owned by the destination core.
  - Layer 1: y = dinv * (x_shard @ W1) on PE (bf16), AllGather -> full
    message table in every core's HBM (the halo exchange).
  - Aggregation (both layers): batched dma_gather fetches 256B bf16 rows
    edge-major; a one-hot ST[128 edges, 128 dst] built on the vector engine
    turns the segment sum into PE matmuls accumulated in PSUM. Self loops
    are added densely from the local shard. The 4 gather streams (one per
    int16-reachable source-table chunk) run on 4 SWDGE queues, i.e. on all
    8 GPSIMD cores in parallel.
  - Layer 2 aggregates h *before* applying W2 (propagation commutes with
    the weight multiply); the @W2 + bias + log_softmax runs per-tile.
  - Each core permutes its nodes into 100 tiles of 128 slots balancing
    per-(tile, chunk) edge counts, so nearly every group packs into 4
    blocks of 128 gathered slots (minimal padding, uniform SPMD streams).
  - Host-side work is integer graph partitioning (shard, bucket, balance,
    bincount for degrees); all float math runs on the NeuronCores.
"""

import heapq
import numpy as np
import ml_dtypes

import concourse.bass as bass
import concourse.bacc as bacc
import concourse.mybir as mybir
import concourse.tile as tile
from concourse import bass_utils
from concourse.masks import make_identity

F32 = mybir.dt.float32
BF16 = mybir.dt.bfloat16
I16 = mybir.dt.int16
AF = mybir.ActivationFunctionType
OP = mybir.AluOpType
NPBF16 = ml_dtypes.bfloat16

N_CORES = 8
N_CHUNKS = 4          # source-table chunks (int16 gather index reach)
P = 128               # partitions / dst-tile size
PAD_DST = 254.0       # dst_local value whose one-hot row is all-zero
SLOT_CAP = 6144       # max gathered edge slots per supertile buffer
TROW = 128            # table row width (bf16): hid data + zero pad = 256B


def _balance_core(d4, nt, cap_nodes=P):
    """Assign nodes (rows of d4 = per-chunk in-degree) to nt tiles,
    balancing per-(tile, chunk) edge loads. Returns slot array [nt*P]
    holding node ids (-1 = empty)."""
    n = d4.shape[0]
    order = np.argsort(-d4.sum(axis=1), kind="stable")
    loads = np.zeros((nt, N_CHUNKS), dtype=np.int64)
    counts = np.zeros(nt, dtype=np.int64)
    slot_of = np.empty(n, dtype=np.int64)
    full_penalty = np.zeros(nt, dtype=np.int64)
    for node in order:
        d = d4[node]
        score = (loads + d).max(axis=1) + full_penalty
        t = int(np.argmin(score))
        loads[t] += d
        slot_of[node] = t * cap_nodes + counts[t]
        counts[t] += 1
        if counts[t] >= cap_nodes:
            full_penalty[t] = 1 << 40
    slots = np.full(nt * cap_nodes, -1, dtype=np.int64)
    slots[slot_of] = np.arange(n)
    return slot_of, slots


class Plan:
    """Static, core-uniform schedule derived from the (integer) graph."""

    def __init__(self, n_nodes, fin, hid, fout, edge_index, n_cores=N_CORES,
                 nt=None):
        assert n_nodes % n_cores == 0
        self.n_nodes = n_nodes
        self.fin, self.hid, self.fout = fin, hid, fout
        self.n_cores = n_cores
        self.base = n_nodes // n_cores
        min_nt = (self.base + P - 1) // P
        self.nt = nt if nt is not None else min_nt + 2 + (min_nt + 63) // 64
        assert self.nt * P >= self.base
        self.nsh = self.nt * P
        self.ntab = self.nsh * n_cores
        assert self.ntab % N_CHUNKS == 0
        self.ch = self.ntab // N_CHUNKS
        assert self.ch <= 32767, "chunk must fit int16 gather index"
        assert fin % P == 0
        self.kch = fin // P

        src = np.asarray(edge_index[0], dtype=np.int64)
        dst = np.asarray(edge_index[1], dtype=np.int64)
        s_owner = src // self.base
        d_owner = dst // self.base
        chunk = s_owner // (n_cores // N_CHUNKS)

        # per-core node balance on per-chunk in-degree
        self.slot_of = np.empty((n_cores, self.base), dtype=np.int64)
        for c in range(n_cores):
            sel = d_owner == c
            dl = dst[sel] - c * self.base
            ck = chunk[sel]
            d4 = np.zeros((self.base, N_CHUNKS), dtype=np.int64)
            np.add.at(d4, (dl, ck), 1)
            slot_of, _ = _balance_core(d4, self.nt)
            self.slot_of[c] = slot_of

        # slot-space edge endpoints
        d_slot = self.slot_of[d_owner, dst - d_owner * self.base]
        gid_src = s_owner * self.nsh + self.slot_of[
            s_owner, src - s_owner * self.base
        ]
        idx_local = gid_src - chunk * self.ch
        tile_id = d_slot // P
        dloc = (d_slot % P).astype(np.float32)

        key = (d_owner * self.nt + tile_id) * N_CHUNKS + chunk
        order = np.argsort(key, kind="stable")
        self._sorted_idx = idx_local[order]
        self._sorted_dloc = dloc[order]
        ngroups = n_cores * self.nt * N_CHUNKS
        sizes = np.bincount(key, minlength=ngroups).reshape(
            n_cores, self.nt, N_CHUNKS
        )
        self._gstart = np.zeros(ngroups + 1, dtype=np.int64)
        np.cumsum(sizes.reshape(-1), out=self._gstart[1:])
        self.sizes = sizes

        nb = (sizes.max(axis=0) + P - 1) // P             # [nt, N_CHUNKS]
        nb[:, 0] = np.maximum(nb[:, 0], 1)
        self.nb = nb
        self.bp = np.zeros((N_CHUNKS, self.nt + 1), dtype=np.int64)
        np.cumsum(nb.T, axis=1, out=self.bp[:, 1:])
        self.blocks_c = self.bp[:, -1].copy()
        self.slots_c = self.blocks_c * P
        self.total_slots = int(self.slots_c.sum())

        self.supertiles = []
        t0 = 0
        while t0 < self.nt:
            t1 = t0 + 1
            while t1 < self.nt:
                tot = int(nb[t0:t1 + 1].sum()) * P
                if tot > SLOT_CAP:
                    break
                t1 += 1
            self.supertiles.append((t0, t1))
            t0 = t1
        self.max_sg_blocks = max(
            int(self.nb[a:b].sum()) for a, b in self.supertiles
        )

        # degrees (with self loop), per core wrapped [128, nt], slot order
        deg = np.bincount(dst, minlength=n_nodes).astype(np.float32) + 1.0
        self.degw = np.ones((n_cores, P, self.nt), dtype=np.float32)
        for c in range(n_cores):
            d = np.ones(self.nsh, dtype=np.float32)
            d[self.slot_of[c]] = deg[c * self.base:(c + 1) * self.base]
            self.degw[c] = d.reshape(self.nt, P).T

        # per-core gather index / dst_local arrays in slot order
        self.idx16 = []
        self.dstl = []
        for c in range(n_cores):
            idx_c, dstl_c = [], []
            for k in range(N_CHUNKS):
                s = int(self.slots_c[k])
                ia = np.zeros(s, dtype=np.int16)
                da = np.full(s, PAD_DST, dtype=np.float32)
                for t in range(self.nt):
                    g = (c * self.nt + t) * N_CHUNKS + k
                    a, b = self._gstart[g], self._gstart[g + 1]
                    o = int(self.bp[k][t]) * P
                    n = int(b - a)
                    ia[o:o + n] = self._sorted_idx[a:b].astype(np.int16)
                    da[o:o + n] = self._sorted_dloc[a:b]
                idx_c.append(np.ascontiguousarray(
                    np.tile(ia.reshape(-1, 16).T, (P // 16, 1))
                ).astype(np.int16))
                dstl_c.append(np.ascontiguousarray(da.reshape(-1, P).T))
            self.idx16.append(idx_c)
            self.dstl.append(dstl_c)


def build_nc(plan: Plan):
    nc = bacc.Bacc(
        "TRN2",
        target_bir_lowering=False,
        debug=False,
        enable_asserts=False,
        num_devices=plan.n_cores,
        num_swdge_queues=N_CHUNKS,
    )
    fin, hid, fout = plan.fin, plan.hid, plan.fout
    nt, nsh, kch = plan.nt, plan.nsh, plan.kch

    xT = nc.dram_tensor("xT", [fin, nsh], BF16, kind="ExternalInput")
    degw = nc.dram_tensor("degw", [P, nt], F32, kind="ExternalInput")
    w1 = nc.dram_tensor("w1", [fin, hid], BF16, kind="ExternalInput")
    b1 = nc.dram_tensor("b1", [1, hid], F32, kind="ExternalInput")
    w2 = nc.dram_tensor("w2", [hid, fout], F32, kind="ExternalInput")
    b2 = nc.dram_tensor("b2", [1, fout], F32, kind="ExternalInput")
    idx_d = [
        nc.dram_tensor(f"idx{c}", [P, int(plan.slots_c[c]) // 16], I16,
                       kind="ExternalInput")
        for c in range(N_CHUNKS)
    ]
    dstl_d = [
        nc.dram_tensor(f"dstl{c}", [P, int(plan.blocks_c[c])], F32,
                       kind="ExternalInput")
        for c in range(N_CHUNKS)
    ]
    out = nc.dram_tensor("out", [nsh, fout], F32, kind="ExternalOutput")

    rg = [list(range(plan.n_cores))]

    with tile.TileContext(nc) as tc:
        with (
            tc.tile_pool(name="const", bufs=1) as cp,
            tc.tile_pool(name="dram", bufs=1, space="DRAM") as dp,
        ):
            # ---- constants -------------------------------------------------
            iota = cp.tile([P, P], BF16)
            nc.gpsimd.iota(iota[:], pattern=[[1, P]], base=0,
                           channel_multiplier=0,
                           allow_small_or_imprecise_dtypes=True)
            ident = cp.tile([P, P], F32)
            make_identity(nc, ident[:])

            w1sb = cp.tile([P, kch, hid], BF16)
            nc.sync.dma_start(
                w1sb[:], w1.ap().rearrange("(a p) f -> p a f", p=P)
            )
            w2sb = cp.tile([hid, fout], F32)
            nc.sync.dma_start(w2sb[:], w2.ap())
            b1row = cp.tile([P, hid], F32)
            nc.sync.dma_start(b1row[:], b1.ap().to_broadcast([P, hid]))
            b2row = cp.tile([P, fout], F32)
            nc.sync.dma_start(b2row[:], b2.ap().to_broadcast([P, fout]))

            degt = cp.tile([P, nt], F32)
            nc.sync.dma_start(degt[:], degw.ap())
            rec = cp.tile([P, nt], F32)
            nc.vector.reciprocal(rec[:], degt[:])
            dinv = cp.tile([P, nt], F32)
            nc.scalar.activation(dinv[:], rec[:], AF.Sqrt)

            idxsb = []
            dstlsb = []
            for c in range(N_CHUNKS):
                it = cp.tile([P, int(plan.slots_c[c]) // 16], I16,
                             tag=f"idx{c}")
                nc.sync.dma_start(it[:], idx_d[c].ap())
                idxsb.append(it)
                dt_ = cp.tile([P, int(plan.blocks_c[c])], F32,
                              tag=f"dstl{c}")
                nc.sync.dma_start(dt_[:], dstl_d[c].ap())
                dstlsb.append(dt_)

            # local table shards (row = TROW bf16: hid data + zero pad)
            w_loc = cp.tile([P, nt, hid], F32)     # dinv*y + b1
            m_all = cp.tile([P, nt], F32)
            ssum_all = cp.tile([P, nt], F32)
            y_loc = cp.tile([P, nt, TROW], BF16)
            z_loc = cp.tile([P, nt, TROW], BF16)
            nc.vector.memset(y_loc[:], 0.0)
            nc.vector.memset(z_loc[:], 0.0)
            out_loc = cp.tile([P, nt, fout], F32)

            y_bounce = dp.tile([nsh, TROW], BF16)
            h_bounce = dp.tile([nsh, TROW], BF16)
            table1 = nc.dram_tensor("table1", [plan.ntab, TROW], BF16,
                                    kind="Internal", addr_space="Shared")
            table2 = nc.dram_tensor("table2", [plan.ntab, TROW], BF16,
                                    kind="Internal", addr_space="Shared")

            # ---- phase 1: y = dinv * (x @ W1), AllGather -> table1 ---------
            WB = 8
            with (
                tc.tile_pool(name="xload", bufs=2) as xp,
                tc.tile_pool(name="ps1", bufs=4, space="PSUM") as pp1,
            ):
                xTap = xT.ap().rearrange("(a p) n -> p a n", p=P)
                for wb in range(0, nt, WB):
                    nwin = min(WB, nt - wb)
                    xt = xp.tile([P, kch, P * WB], BF16, tag="xt")
                    nc.sync.dma_start(
                        xt[:, :, : P * nwin],
                        xTap[:, :, wb * P:(wb + nwin) * P],
                    )
                    for w in range(nwin):
                        t = wb + w
                        ps = pp1.tile([P, hid], F32, tag="ps1")
                        for a in range(kch):
                            nc.tensor.matmul(
                                ps[:],
                                lhsT=xt[:, a, w * P:(w + 1) * P],
                                rhs=w1sb[:, a, :],
                                start=(a == 0),
                                stop=(a == kch - 1),
                            )
                        nc.vector.tensor_scalar(
                            out=y_loc[:, t, :hid], in0=ps[:],
                            scalar1=dinv[:, t:t + 1], scalar2=None,
                            op0=OP.mult,
                        )
                        nc.vector.scalar_tensor_tensor(
                            out=w_loc[:, t, :], in0=y_loc[:, t, :hid],
                            scalar=dinv[:, t:t + 1], in1=b1row[:],
                            op0=OP.mult, op1=OP.add,
                        )
            nc.sync.dma_start(
                y_bounce[:].rearrange("(t p) f -> p t f", p=P), y_loc[:]
            )
            nc.gpsimd.collective_compute(
                "AllGather", OP.bypass, replica_groups=rg,
                ins=[y_bounce.opt()], outs=[table1.ap()],
            )

            # ---- aggregation pass (both layers) ----------------------------
            def aggregate(table, epilogue):
                with (
                    tc.tile_pool(name="gath", bufs=2) as gp,
                    tc.tile_pool(name="stp", bufs=4) as stp,
                    tc.tile_pool(name="ps2", bufs=3, space="PSUM") as pp2,
                    tc.tile_pool(name="eps", bufs=3) as ep,
                    tc.tile_pool(name="psT", bufs=2, space="PSUM") as ppT,
                    tc.tile_pool(name="pso", bufs=2, space="PSUM") as ppo,
                ):
                    for (t0, t1) in plan.supertiles:
                        off = {}
                        yb = gp.tile([P, plan.max_sg_blocks, TROW], BF16,
                                     tag="yb")
                        o = 0
                        for c in range(N_CHUNKS):
                            blk0 = int(plan.bp[c][t0])
                            blk1 = int(plan.bp[c][t1])
                            nbg = blk1 - blk0
                            off[c] = (o, blk0)
                            if nbg == 0:
                                continue
                            nc.gpsimd.dma_gather(
                                yb[:, o:o + nbg, :],
                                table.ap()[c * plan.ch:(c + 1) * plan.ch, :],
                                idxsb[c][:, blk0 * 8:blk1 * 8],
                                nbg * P,
                                nbg * P,
                                TROW,
                                single_packet=False,
                                queue_num=c,
                            )
                            o += nbg
                        for t in range(t0, t1):
                            ps = pp2.tile([P, hid], F32, tag="ps2")
                            total = int(plan.nb[t].sum())
                            sts = {}
                            for c in range(N_CHUNKS):
                                nbt = int(plan.nb[t][c])
                                if nbt == 0:
                                    continue
                                b0 = int(plan.bp[c][t])
                                st = stp.tile([P, nbt, P], BF16, tag="st",
                                              name=f"st{c}")
                                nc.vector.tensor_tensor(
                                    out=st[:],
                                    in0=iota[:].rearrange(
                                        "p (a f) -> p a f", a=1
                                    ).to_broadcast([P, nbt, P]),
                                    in1=dstlsb[c][:, b0:b0 + nbt].rearrange(
                                        "p (b o) -> p b o", o=1
                                    ).to_broadcast([P, nbt, P]),
                                    op=OP.is_equal,
                                )
                                sts[c] = st
                            done = 0
                            for c in range(N_CHUNKS):
                                o, blk0 = off[c]
                                b0 = int(plan.bp[c][t])
                                for b in range(b0, int(plan.bp[c][t + 1])):
                                    nc.tensor.matmul(
                                        ps[:], lhsT=sts[c][:, b - b0, :],
                                        rhs=yb[:, o + (b - blk0), :hid],
                                        start=(done == 0),
                                        stop=(done == total - 1),
                                    )
                                    done += 1
                            epilogue(t, ps, ep, ppT, ppo)

            # ---- layer-1 epilogue: z = dinv*relu(dinv*(s+y) + b1) ----------
            def epi1(t, ps, ep, ppT, ppo):
                a2 = ep.tile([P, hid], F32, tag="a2")
                nc.vector.scalar_tensor_tensor(
                    out=a2[:], in0=ps[:], scalar=dinv[:, t:t + 1],
                    in1=w_loc[:, t, :], op0=OP.mult, op1=OP.add,
                )
                nc.vector.tensor_scalar(
                    out=z_loc[:, t, :hid], in0=a2[:],
                    scalar1=dinv[:, t:t + 1], scalar2=0.0,
                    op0=OP.mult, op1=OP.max,
                )

            aggregate(table1, epi1)
            nc.sync.dma_start(
                h_bounce[:].rearrange("(t p) f -> p t f", p=P), z_loc[:]
            )
            nc.gpsimd.collective_compute(
                "AllGather", OP.bypass, replica_groups=rg,
                ins=[h_bounce.opt()], outs=[table2.ap()],
            )

            # ---- layer-2 epilogue: log_softmax(dinv*(s+z) @ W2 + b2) -------
            def epi2(t, ps, ep, ppT, ppo):
                u = ep.tile([P, hid], F32, tag="u")
                nc.vector.tensor_tensor(
                    out=u[:], in0=ps[:], in1=z_loc[:, t, :hid], op=OP.add
                )
                opre = ep.tile([P, hid], F32, tag="a1")
                nc.vector.tensor_scalar(
                    out=opre[:], in0=u[:], scalar1=dinv[:, t:t + 1],
                    scalar2=None, op0=OP.mult,
                )
                pT = ppT.tile([hid, P], F32, tag="pT")
                nc.tensor.transpose(out=pT[:], in_=opre[:],
                                    identity=ident[:])
                opT = ep.tile([hid, P], F32, tag="opT")
                nc.scalar.copy(opT[:], pT[:])
                po = ppo.tile([P, fout], F32, tag="po")
                nc.tensor.matmul(po[:], lhsT=opT[:], rhs=w2sb[:],
                                 start=True, stop=True)
                nc.vector.tensor_tensor(
                    out=out_loc[:, t, :], in0=po[:], in1=b2row[:], op=OP.add
                )
                nc.vector.reduce_max(m_all[:, t:t + 1], out_loc[:, t, :],
                                     axis=mybir.AxisListType.X, negate=True)
                e = ep.tile([P, fout], F32, tag="e")
                nc.scalar.activation(e[:], out_loc[:, t, :], AF.Exp,
                                     bias=m_all[:, t:t + 1],
                                     accum_out=ssum_all[:, t:t + 1])

            aggregate(table2, epi2)
            # deferred log-sum-exp: out -= log(ssum) - m_all (m_all = -max)
            lse_all = cp.tile([P, nt], F32)
            nc.scalar.activation(lse_all[:], ssum_all[:], AF.Ln)
            c_all = cp.tile([P, nt], F32)
            nc.vector.tensor_tensor(
                out=c_all[:], in0=lse_all[:], in1=m_all[:], op=OP.subtract
            )
            with tc.tile_pool(name="fin", bufs=4) as fp:
                for t in range(nt):
                    ot = fp.tile([P, fout], F32, tag="ot")
                    nc.vector.tensor_scalar(
                        out=ot[:], in0=out_loc[:, t, :],
                        scalar1=c_all[:, t:t + 1], scalar2=None,
                        op0=OP.subtract,
                    )
                    nc.vector.tensor_copy(out_loc[:, t, :], ot[:])
            nc.sync.dma_start(
                out.ap().rearrange("(t p) f -> p t f", p=P), out_loc[:]
            )

    nc.compile()
    return nc


def make_in_maps(plan: Plan, x, W1, b1, W2, b2):
    x = np.asarray(x, dtype=np.float32)
    w1b = np.ascontiguousarray(W1, dtype=np.float32).astype(NPBF16)
    in_maps = []
    for c in range(plan.n_cores):
        xT = np.zeros((plan.fin, plan.nsh), dtype=NPBF16)
        xs = x[c * plan.base:(c + 1) * plan.base, :].astype(NPBF16)
        xT[:, plan.slot_of[c]] = xs.T
        m = {
            "xT": xT,
            "degw": plan.degw[c],
            "w1": w1b,
            "b1": np.asarray(b1, dtype=np.float32).reshape(1, -1),
            "w2": np.ascontiguousarray(W2, dtype=np.float32),
            "b2": np.asarray(b2, dtype=np.float32).reshape(1, -1),
        }
        for k in range(N_CHUNKS):
            m[f"idx{k}"] = plan.idx16[c][k]
            m[f"dstl{k}"] = plan.dstl[c][k]
        in_maps.append(m)
    return in_maps


_CACHE = {}


def _get_compiled(n_nodes, fin, hid, fout, edge_key, edge_index):
    key = (n_nodes, fin, hid, fout, edge_key)
    if key not in _CACHE:
        plan = Plan(n_nodes, fin, hid, fout, edge_index)
        nc = build_nc(plan)
        _CACHE[key] = (plan, nc)
    return _CACHE[key]


def kernel(x, edge_index, W1, b1, W2, b2, _trace=False):
    x = np.asarray(x)
    edge_index = np.asarray(edge_index)
    n_nodes, fin = x.shape
    hid = np.asarray(W1).shape[1]
    fout = np.asarray(W2).shape[1]
    edge_key = hash(edge_index.tobytes())
    plan, nc = _get_compiled(n_nodes, fin, hid, fout, edge_key, edge_index)
    in_maps = make_in_maps(plan, x, W1, b1, W2, b2)
    res = bass_utils.run_bass_kernel_spmd(
        nc, in_maps, core_ids=list(range(plan.n_cores)), trace=_trace
    )
    parts = [
        res.results[c]["out"][plan.slot_of[c], :]
        for c in range(plan.n_cores)
    ]
    out = np.concatenate(parts, axis=0).astype(np.float32)
    kernel.last_results = res
    return out


# revision 10
# speedup vs baseline: 1.8792x; 1.2015x over previous
"""Two-layer GCN (nn_Method_GCN_11098195493080) as a Bass/Tile kernel on 8
Trainium2 NeuronCores.

Strategy (follows the 1D graph-partition sharding hint):
  - Nodes sharded 8 ways; edges owned by the destination core.
  - Layer 1: y = dinv * (x_shard @ W1) on PE (bf16), AllGather -> full
    message table in every core's HBM (the halo exchange).
  - Aggregation (both layers): batched dma_gather fetches 256B bf16 rows
    edge-major; a one-hot ST[128 edges, 128 dst] built on the vector engine
    turns the segment sum into PE matmuls accumulated in PSUM. Self loops
    are added densely from the local shard. The 4 gather streams (one per
    int16-reachable source-table chunk) run on 4 SWDGE queues, i.e. on all
    8 GPSIMD cores in parallel.
  - Layer 2 aggregates h *before* applying W2 (propagation commutes with
    the weight multiply); the @W2 + bias + log_softmax runs per-tile.
  - Each core permutes its nodes into 100 tiles of 128 slots balancing
    per-(tile, chunk) edge counts, so nearly every group packs into 4
    blocks of 128 gathered slots (minimal padding, uniform SPMD streams).
  - Host-side work is integer graph partitioning (shard, bucket, balance,
    bincount for degrees); all float math runs on the NeuronCores.
"""

import heapq
import numpy as np
import ml_dtypes

import concourse.bass as bass
import concourse.bacc as bacc
import concourse.mybir as mybir
import concourse.tile as tile
from concourse import bass_utils
from concourse.masks import make_identity

F32 = mybir.dt.float32
BF16 = mybir.dt.bfloat16
I16 = mybir.dt.int16
AF = mybir.ActivationFunctionType
OP = mybir.AluOpType
NPBF16 = ml_dtypes.bfloat16

N_CORES = 8
N_CHUNKS = 4          # source-table chunks (int16 gather index reach)
P = 128               # partitions / dst-tile size
PAD_DST = 254.0       # dst_local value whose one-hot row is all-zero
SLOT_CAP = 6144       # max gathered edge slots per supertile buffer
TROW = 128            # table row width (bf16): hid data + zero pad = 256B


def _balance_core(d4, nt, cap_nodes=P):
    """Assign nodes (rows of d4 = per-chunk in-degree) to nt tiles,
    balancing per-(tile, chunk) edge loads. Returns slot array [nt*P]
    holding node ids (-1 = empty)."""
    n = d4.shape[0]
    order = np.argsort(-d4.sum(axis=1), kind="stable")
    loads = np.zeros((nt, N_CHUNKS), dtype=np.int64)
    counts = np.zeros(nt, dtype=np.int64)
    slot_of = np.empty(n, dtype=np.int64)
    full_penalty = np.zeros(nt, dtype=np.int64)
    for node in order:
        d = d4[node]
        score = (loads + d).max(axis=1) + full_penalty
        t = int(np.argmin(score))
        loads[t] += d
        slot_of[node] = t * cap_nodes + counts[t]
        counts[t] += 1
        if counts[t] >= cap_nodes:
            full_penalty[t] = 1 << 40
    slots = np.full(nt * cap_nodes, -1, dtype=np.int64)
    slots[slot_of] = np.arange(n)
    return slot_of, slots


class Plan:
    """Static, core-uniform schedule derived from the (integer) graph."""

    def __init__(self, n_nodes, fin, hid, fout, edge_index, n_cores=N_CORES,
                 nt=None):
        assert n_nodes % n_cores == 0
        self.n_nodes = n_nodes
        self.fin, self.hid, self.fout = fin, hid, fout
        self.n_cores = n_cores
        self.base = n_nodes // n_cores
        min_nt = (self.base + P - 1) // P
        self.nt = nt if nt is not None else min_nt + 2 + (min_nt + 63) // 64
        assert self.nt * P >= self.base
        self.nsh = self.nt * P
        self.ntab = self.nsh * n_cores
        assert self.ntab % N_CHUNKS == 0
        self.ch = self.ntab // N_CHUNKS
        assert self.ch <= 32767, "chunk must fit int16 gather index"
        assert fin % P == 0
        self.kch = fin // P

        src = np.asarray(edge_index[0], dtype=np.int64)
        dst = np.asarray(edge_index[1], dtype=np.int64)
        s_owner = src // self.base
        d_owner = dst // self.base
        chunk = s_owner // (n_cores // N_CHUNKS)

        # per-core node balance on per-chunk in-degree
        self.slot_of = np.empty((n_cores, self.base), dtype=np.int64)
        for c in range(n_cores):
            sel = d_owner == c
            dl = dst[sel] - c * self.base
            ck = chunk[sel]
            d4 = np.zeros((self.base, N_CHUNKS), dtype=np.int64)
            np.add.at(d4, (dl, ck), 1)
            slot_of, _ = _balance_core(d4, self.nt)
            self.slot_of[c] = slot_of

        # slot-space edge endpoints; table rows are partition-major:
        # gid = core*nsh + p*nt + t  (slot_of stores t*128+p)
        def pmaj(tp):
            return (tp % P) * self.nt + tp // P
        self._pmaj = pmaj
        d_slot = self.slot_of[d_owner, dst - d_owner * self.base]
        tp_src = self.slot_of[s_owner, src - s_owner * self.base]
        gid_src = s_owner * self.nsh + pmaj(tp_src)
        idx_local = gid_src - chunk * self.ch
        tile_id = d_slot // P
        dloc = (d_slot % P).astype(np.float32)

        key = (d_owner * self.nt + tile_id) * N_CHUNKS + chunk
        order = np.argsort(key, kind="stable")
        self._sorted_idx = idx_local[order]
        self._sorted_dloc = dloc[order]
        ngroups = n_cores * self.nt * N_CHUNKS
        sizes = np.bincount(key, minlength=ngroups).reshape(
            n_cores, self.nt, N_CHUNKS
        )
        self._gstart = np.zeros(ngroups + 1, dtype=np.int64)
        np.cumsum(sizes.reshape(-1), out=self._gstart[1:])
        self.sizes = sizes

        nb = (sizes.max(axis=0) + P - 1) // P             # [nt, N_CHUNKS]
        nb[:, 0] = np.maximum(nb[:, 0], 1)
        self.nb = nb
        self.bp = np.zeros((N_CHUNKS, self.nt + 1), dtype=np.int64)
        np.cumsum(nb.T, axis=1, out=self.bp[:, 1:])
        self.blocks_c = self.bp[:, -1].copy()
        self.slots_c = self.blocks_c * P
        self.total_slots = int(self.slots_c.sum())

        self.supertiles = []
        t0 = 0
        while t0 < self.nt:
            t1 = t0 + 1
            while t1 < self.nt:
                tot = int(nb[t0:t1 + 1].sum()) * P
                if tot > SLOT_CAP:
                    break
                t1 += 1
            self.supertiles.append((t0, t1))
            t0 = t1
        self.max_sg_blocks = max(
            int(self.nb[a:b].sum()) for a, b in self.supertiles
        )

        # degrees (with self loop), per core wrapped [128, nt], slot order
        deg = np.bincount(dst, minlength=n_nodes).astype(np.float32) + 1.0
        self.degw = np.ones((n_cores, P, self.nt), dtype=np.float32)
        for c in range(n_cores):
            d = np.ones(self.nsh, dtype=np.float32)
            d[self.slot_of[c]] = deg[c * self.base:(c + 1) * self.base]
            self.degw[c] = d.reshape(self.nt, P).T

        # per-core gather index / dst_local arrays in slot order
        self.idx16 = []
        self.dstl = []
        for c in range(n_cores):
            idx_c, dstl_c = [], []
            for k in range(N_CHUNKS):
                s = int(self.slots_c[k])
                ia = np.zeros(s, dtype=np.int16)
                da = np.full(s, PAD_DST, dtype=np.float32)
                for t in range(self.nt):
                    g = (c * self.nt + t) * N_CHUNKS + k
                    a, b = self._gstart[g], self._gstart[g + 1]
                    o = int(self.bp[k][t]) * P
                    n = int(b - a)
                    ia[o:o + n] = self._sorted_idx[a:b].astype(np.int16)
                    da[o:o + n] = self._sorted_dloc[a:b]
                idx_c.append(np.ascontiguousarray(
                    np.tile(ia.reshape(-1, 16).T, (P // 16, 1))
                ).astype(np.int16))
                dstl_c.append(np.ascontiguousarray(da.reshape(-1, P).T))
            self.idx16.append(idx_c)
            self.dstl.append(dstl_c)


def build_nc(plan: Plan):
    nc = bacc.Bacc(
        "TRN2",
        target_bir_lowering=False,
        debug=False,
        enable_asserts=False,
        num_devices=plan.n_cores,
        num_swdge_queues=N_CHUNKS,
    )
    fin, hid, fout = plan.fin, plan.hid, plan.fout
    nt, nsh, kch = plan.nt, plan.nsh, plan.kch

    xT = nc.dram_tensor("xT", [fin, nsh], BF16, kind="ExternalInput")
    degw = nc.dram_tensor("degw", [P, nt], F32, kind="ExternalInput")
    w1 = nc.dram_tensor("w1", [fin, hid], BF16, kind="ExternalInput")
    b1 = nc.dram_tensor("b1", [1, hid], F32, kind="ExternalInput")
    w2 = nc.dram_tensor("w2", [hid, fout], F32, kind="ExternalInput")
    b2 = nc.dram_tensor("b2", [1, fout], F32, kind="ExternalInput")
    idx_d = [
        nc.dram_tensor(f"idx{c}", [P, int(plan.slots_c[c]) // 16], I16,
                       kind="ExternalInput")
        for c in range(N_CHUNKS)
    ]
    dstl_d = [
        nc.dram_tensor(f"dstl{c}", [P, int(plan.blocks_c[c])], F32,
                       kind="ExternalInput")
        for c in range(N_CHUNKS)
    ]
    out = nc.dram_tensor("out", [nsh, fout], F32, kind="ExternalOutput")

    rg = [list(range(plan.n_cores))]

    with tile.TileContext(nc) as tc:
        with (
            tc.tile_pool(name="const", bufs=1) as cp,
            tc.tile_pool(name="dram", bufs=1, space="DRAM") as dp,
        ):
            # ---- constants -------------------------------------------------
            iota = cp.tile([P, P], BF16)
            nc.gpsimd.iota(iota[:], pattern=[[1, P]], base=0,
                           channel_multiplier=0,
                           allow_small_or_imprecise_dtypes=True)
            ident = cp.tile([P, P], F32)
            make_identity(nc, ident[:])

            w1sb = cp.tile([P, kch, hid], BF16)
            nc.sync.dma_start(
                w1sb[:], w1.ap().rearrange("(a p) f -> p a f", p=P)
            )
            w2sb = cp.tile([hid, fout], F32)
            nc.sync.dma_start(w2sb[:], w2.ap())
            b1row = cp.tile([P, hid], F32)
            nc.sync.dma_start(b1row[:], b1.ap().to_broadcast([P, hid]))
            b2row = cp.tile([P, fout], F32)
            nc.sync.dma_start(b2row[:], b2.ap().to_broadcast([P, fout]))

            degt = cp.tile([P, nt], F32)
            nc.sync.dma_start(degt[:], degw.ap())
            rec = cp.tile([P, nt], F32)
            nc.vector.reciprocal(rec[:], degt[:])
            dinv = cp.tile([P, nt], F32)
            nc.scalar.activation(dinv[:], rec[:], AF.Sqrt)

            idxsb = []
            dstlsb = []
            for c in range(N_CHUNKS):
                it = cp.tile([P, int(plan.slots_c[c]) // 16], I16,
                             tag=f"idx{c}")
                nc.sync.dma_start(it[:], idx_d[c].ap())
                idxsb.append(it)
                dt_ = cp.tile([P, int(plan.blocks_c[c])], F32,
                              tag=f"dstl{c}")
                nc.sync.dma_start(dt_[:], dstl_d[c].ap())
                dstlsb.append(dt_)

            # local table shards (row = TROW bf16: hid data + zero pad)
            w_loc = cp.tile([P, nt, hid], F32)     # dinv*y + b1
            m_all = cp.tile([P, nt], F32)
            ssum_all = cp.tile([P, nt], F32)
            y_loc = cp.tile([P, nt, TROW], BF16)
            z_loc = cp.tile([P, nt, TROW], BF16)
            nc.vector.memset(y_loc[:], 0.0)
            nc.vector.memset(z_loc[:], 0.0)
            out_loc = cp.tile([P, nt, fout], F32)

            y_bounce = dp.tile([nsh, TROW], BF16)
            h_bounce = dp.tile([nsh, TROW], BF16)
            table1 = nc.dram_tensor("table1", [plan.ntab, TROW], BF16,
                                    kind="Internal", addr_space="Shared")
            table2 = nc.dram_tensor("table2", [plan.ntab, TROW], BF16,
                                    kind="Internal", addr_space="Shared")

            # ---- phase 1: y = dinv * (x @ W1), AllGather -> table1 ---------
            WB = 8
            with (
                tc.tile_pool(name="xload", bufs=2) as xp,
                tc.tile_pool(name="ps1", bufs=4, space="PSUM") as pp1,
            ):
                xTap = xT.ap().rearrange("(a p) n -> p a n", p=P)
                for wb in range(0, nt, WB):
                    nwin = min(WB, nt - wb)
                    xt = xp.tile([P, kch, P * WB], BF16, tag="xt")
                    nc.sync.dma_start(
                        xt[:, :, : P * nwin],
                        xTap[:, :, wb * P:(wb + nwin) * P],
                    )
                    for w in range(nwin):
                        t = wb + w
                        ps = pp1.tile([P, hid], F32, tag="ps1")
                        for a in range(kch):
                            nc.tensor.matmul(
                                ps[:],
                                lhsT=xt[:, a, w * P:(w + 1) * P],
                                rhs=w1sb[:, a, :],
                                start=(a == 0),
                                stop=(a == kch - 1),
                            )
                        nc.vector.tensor_scalar(
                            out=y_loc[:, t, :hid], in0=ps[:],
                            scalar1=dinv[:, t:t + 1], scalar2=None,
                            op0=OP.mult,
                        )
                        nc.vector.scalar_tensor_tensor(
                            out=w_loc[:, t, :], in0=y_loc[:, t, :hid],
                            scalar=dinv[:, t:t + 1], in1=b1row[:],
                            op0=OP.mult, op1=OP.add,
                        )
            nc.sync.dma_start(
                y_bounce[:].rearrange("(p t) f -> p t f", p=P), y_loc[:]
            )
            nc.gpsimd.collective_compute(
                "AllGather", OP.bypass, replica_groups=rg,
                ins=[y_bounce.opt()], outs=[table1.ap()],
            )

            # ---- aggregation pass (both layers) ----------------------------
            def aggregate(table, epilogue):
                with (
                    tc.tile_pool(name="gath", bufs=2) as gp,
                    tc.tile_pool(name="stp", bufs=4) as stp,
                    tc.tile_pool(name="ps2", bufs=3, space="PSUM") as pp2,
                    tc.tile_pool(name="eps", bufs=3) as ep,
                    tc.tile_pool(name="psT", bufs=2, space="PSUM") as ppT,
                    tc.tile_pool(name="pso", bufs=2, space="PSUM") as ppo,
                ):
                    for (t0, t1) in plan.supertiles:
                        off = {}
                        yb = gp.tile([P, plan.max_sg_blocks, TROW], BF16,
                                     tag="yb")
                        o = 0
                        for c in range(N_CHUNKS):
                            blk0 = int(plan.bp[c][t0])
                            blk1 = int(plan.bp[c][t1])
                            nbg = blk1 - blk0
                            off[c] = (o, blk0)
                            if nbg == 0:
                                continue
                            nc.gpsimd.dma_gather(
                                yb[:, o:o + nbg, :],
                                table.ap()[c * plan.ch:(c + 1) * plan.ch, :],
                                idxsb[c][:, blk0 * 8:blk1 * 8],
                                nbg * P,
                                nbg * P,
                                TROW,
                                single_packet=False,
                                queue_num=c,
                            )
                            o += nbg
                        for t in range(t0, t1):
                            ps = pp2.tile([P, hid], F32, tag="ps2")
                            total = int(plan.nb[t].sum())
                            sts = {}
                            for c in range(N_CHUNKS):
                                nbt = int(plan.nb[t][c])
                                if nbt == 0:
                                    continue
                                b0 = int(plan.bp[c][t])
                                st = stp.tile([P, nbt, P], BF16, tag="st",
                                              name=f"st{c}")
                                nc.vector.tensor_tensor(
                                    out=st[:],
                                    in0=iota[:].rearrange(
                                        "p (a f) -> p a f", a=1
                                    ).to_broadcast([P, nbt, P]),
                                    in1=dstlsb[c][:, b0:b0 + nbt].rearrange(
                                        "p (b o) -> p b o", o=1
                                    ).to_broadcast([P, nbt, P]),
                                    op=OP.is_equal,
                                )
                                sts[c] = st
                            done = 0
                            for c in range(N_CHUNKS):
                                o, blk0 = off[c]
                                b0 = int(plan.bp[c][t])
                                for b in range(b0, int(plan.bp[c][t + 1])):
                                    nc.tensor.matmul(
                                        ps[:], lhsT=sts[c][:, b - b0, :],
                                        rhs=yb[:, o + (b - blk0), :hid],
                                        start=(done == 0),
                                        stop=(done == total - 1),
                                    )
                                    done += 1
                            epilogue(t, ps, ep, ppT, ppo)

            # ---- layer-1 epilogue: z = dinv*relu(dinv*(s+y) + b1) ----------
            def epi1(t, ps, ep, ppT, ppo):
                a2 = ep.tile([P, hid], F32, tag="a2")
                nc.vector.scalar_tensor_tensor(
                    out=a2[:], in0=ps[:], scalar=dinv[:, t:t + 1],
                    in1=w_loc[:, t, :], op0=OP.mult, op1=OP.add,
                )
                nc.scalar.activation(
                    z_loc[:, t, :hid], a2[:], AF.Relu,
                    scale=dinv[:, t:t + 1],
                )

            aggregate(table1, epi1)
            nc.sync.dma_start(
                h_bounce[:].rearrange("(p t) f -> p t f", p=P), z_loc[:]
            )
            nc.gpsimd.collective_compute(
                "AllGather", OP.bypass, replica_groups=rg,
                ins=[h_bounce.opt()], outs=[table2.ap()],
            )

            # ---- layer-2 epilogue: log_softmax(dinv*(s+z) @ W2 + b2) -------
            def epi2(t, ps, ep, ppT, ppo):
                u = ep.tile([P, hid], F32, tag="u")
                nc.vector.tensor_tensor(
                    out=u[:], in0=ps[:], in1=z_loc[:, t, :hid], op=OP.add
                )
                opre = ep.tile([P, hid], F32, tag="a1")
                nc.scalar.mul(opre[:], u[:], dinv[:, t:t + 1])
                pT = ppT.tile([hid, P], F32, tag="pT")
                nc.tensor.transpose(out=pT[:], in_=opre[:],
                                    identity=ident[:])
                opT = ep.tile([hid, P], F32, tag="opT")
                nc.scalar.copy(opT[:], pT[:])
                po = ppo.tile([P, fout], F32, tag="po")
                nc.tensor.matmul(po[:], lhsT=opT[:], rhs=w2sb[:],
                                 start=True, stop=True)
                nc.vector.tensor_tensor(
                    out=out_loc[:, t, :], in0=po[:], in1=b2row[:], op=OP.add
                )
                nc.vector.reduce_max(m_all[:, t:t + 1], out_loc[:, t, :],
                                     axis=mybir.AxisListType.X, negate=True)
                e = ep.tile([P, fout], F32, tag="e")
                nc.scalar.activation(e[:], out_loc[:, t, :], AF.Exp,
                                     bias=m_all[:, t:t + 1],
                                     accum_out=ssum_all[:, t:t + 1])

            aggregate(table2, epi2)
            # deferred log-sum-exp: out -= log(ssum) - m_all (m_all = -max)
            lse_all = cp.tile([P, nt], F32)
            nc.scalar.activation(lse_all[:], ssum_all[:], AF.Ln)
            c_all = cp.tile([P, nt], F32)
            nc.vector.tensor_tensor(
                out=c_all[:], in0=lse_all[:], in1=m_all[:], op=OP.subtract
            )
            for t in range(nt):
                nc.vector.tensor_scalar(
                    out=out_loc[:, t, :], in0=out_loc[:, t, :],
                    scalar1=c_all[:, t:t + 1], scalar2=None,
                    op0=OP.subtract,
                )
            nc.sync.dma_start(
                out.ap().rearrange("(p t) f -> p t f", p=P), out_loc[:]
            )

    nc.compile()
    return nc


def make_in_maps(plan: Plan, x, W1, b1, W2, b2):
    x = np.asarray(x, dtype=np.float32)
    w1b = np.ascontiguousarray(W1, dtype=np.float32).astype(NPBF16)
    in_maps = []
    for c in range(plan.n_cores):
        xT = np.zeros((plan.fin, plan.nsh), dtype=NPBF16)
        xs = x[c * plan.base:(c + 1) * plan.base, :].astype(NPBF16)
        xT[:, plan.slot_of[c]] = xs.T
        m = {
            "xT": xT,
            "degw": plan.degw[c],
            "w1": w1b,
            "b1": np.asarray(b1, dtype=np.float32).reshape(1, -1),
            "w2": np.ascontiguousarray(W2, dtype=np.float32),
            "b2": np.asarray(b2, dtype=np.float32).reshape(1, -1),
        }
        for k in range(N_CHUNKS):
            m[f"idx{k}"] = plan.idx16[c][k]
            m[f"dstl{k}"] = plan.dstl[c][k]
        in_maps.append(m)
    return in_maps


_CACHE = {}


def _get_compiled(n_nodes, fin, hid, fout, edge_key, edge_index):
    key = (n_nodes, fin, hid, fout, edge_key)
    if key not in _CACHE:
        plan = Plan(n_nodes, fin, hid, fout, edge_index)
        nc = build_nc(plan)
        _CACHE[key] = (plan, nc)
    return _CACHE[key]


def kernel(x, edge_index, W1, b1, W2, b2, _trace=False):
    x = np.asarray(x)
    edge_index = np.asarray(edge_index)
    n_nodes, fin = x.shape
    hid = np.asarray(W1).shape[1]
    fout = np.asarray(W2).shape[1]
    edge_key = hash(edge_index.tobytes())
    plan, nc = _get_compiled(n_nodes, fin, hid, fout, edge_key, edge_index)
    in_maps = make_in_maps(plan, x, W1, b1, W2, b2)
    res = bass_utils.run_bass_kernel_spmd(
        nc, in_maps, core_ids=list(range(plan.n_cores)), trace=_trace
    )
    parts = [
        res.results[c]["out"][plan._pmaj(plan.slot_of[c]), :]
        for c in range(plan.n_cores)
    ]
    out = np.concatenate(parts, axis=0).astype(np.float32)
    kernel.last_results = res
    return out
